# revision 14
# baseline (speedup 1.0000x reference)
"""Single-head full attention (B=4, S=4096, D=512) on 8 TRN2 NeuronCores.

Sharding: core c handles batch b = c//2, query half h = c%2 (2048 queries).
Each core projects K/V for its batch's full sequence (recompute across the
core pair — cheaper than a collective), projects its query half, then runs
a flash-style attention entirely on-chip.

Device layouts (per core):
  xT   [512, 4096]  = x[b].T           (host pre-transposed)
  xqT  [512, 2048]  = query-half cols of xT
  wqT/wkT/wvT [512, 512] = W.T         (host pre-transposed)
  y    [2048, 512]  = output rows for this core's query half

On-chip (partition index p, 128 partitions):
  kt_sb [128, 4, 4096] f32 : K^T, partition p + tile me -> e = me*128+p
  qt_sb [128, 4, 2048] f32 : Q^T, same e layout
  v_sb  [128, 32, 512] bf16: V natural, partition p + block jb -> j = jb*128+p
Scores are computed transposed (S^T[j, q]) so exp(S^T) blocks serve directly
as the stationary operand of the P@V matmul, producing O in natural [q, d]
orientation; softmax denominators come from an N=1 ones-matmul sharing the
same stationary tile. Matmuls run as float32r (full PE rate at N>=512) for
projections and scores; P@V runs bf16. exp is scalar-engine with the 1/sqrt(D)
scale fused; no max-subtraction (scores are provably O(1) for this problem:
softmax is shift-invariant, exp stays in fp32 range).
"""
import math
import os
import numpy as np

B, S, D = 4, 4096, 512
P = 128
SQ = S // 2          # queries per core
NCORES = 8
QTILE = 512          # query columns per score/PV pass
CH = 256             # projection chunk (sequence columns per x chunk)

last_results = None  # BassKernelResults of the most recent run (for test.py)

_nc_cache = {}


def _build_nc(has_bias, has_mask, reps=1):
    import concourse.bacc as bacc
    import concourse.tile as tile
    from concourse import mybir
    from contextlib import ExitStack

    f32 = mybir.dt.float32
    f32r = mybir.dt.float32r
    bf16 = mybir.dt.bfloat16
    Exp = mybir.ActivationFunctionType.Exp

    nc = bacc.Bacc("TRN2", target_bir_lowering=False, debug=False)
    xT = nc.declare_dram_parameter("xT", [D, S], f32r, False)
    xqT = nc.declare_dram_parameter("xqT", [D, SQ], f32r, False)
    wqT = nc.declare_dram_parameter("wqT", [D, D], f32r, False)
    wkT = nc.declare_dram_parameter("wkT", [D, D], f32r, False)
    wvT = nc.declare_dram_parameter("wvT", [D, D], f32r, False)
    if has_bias:
        bq2 = nc.declare_dram_parameter("bq2", [P, D // P], f32, False)
        bk2 = nc.declare_dram_parameter("bk2", [P, D // P], f32, False)
        bvr = nc.declare_dram_parameter("bvr", [P, D], f32, False)
    if has_mask:
        maskf = nc.declare_dram_parameter("maskf", [P, S // P], f32, False)
    y = nc.declare_dram_parameter("y", [SQ, D], f32, True)

    ET = D // P          # 4 e-tiles
    NJB = S // P         # 32 key blocks
    NQT = SQ // QTILE    # 4 query tiles
    NQS = QTILE // P     # 4 query subblocks per tile
    inv_sqrt_d = 1.0 / math.sqrt(D)

    with tile.TileContext(nc) as tc, ExitStack() as ctx:
        wpool = ctx.enter_context(tc.tile_pool(name="wpool", bufs=1))
        big = ctx.enter_context(tc.tile_pool(name="big", bufs=1))
        xin = ctx.enter_context(tc.tile_pool(name="xin", bufs=2 if (has_bias or has_mask) else 3))
        expp = ctx.enter_context(tc.tile_pool(name="expp", bufs=3))
        outp = ctx.enter_context(tc.tile_pool(name="outp", bufs=2))
        smallp = ctx.enter_context(tc.tile_pool(name="smallp", bufs=2))
        # PSUM: one shared [128,512] accumulate tag (projections + scores)
        # keeps every phase inside 8 banks: 3 (mm512) + 4 (po) + 1 (sums).
        psum_mm = ctx.enter_context(tc.tile_pool(name="psum_mm", bufs=3, space="PSUM"))
        psum_o = ctx.enter_context(tc.tile_pool(name="psum_o", bufs=1, space="PSUM"))
        psum_sum = ctx.enter_context(tc.tile_pool(name="psum_sum", bufs=1, space="PSUM"))

        wq_sb = wpool.tile([P, ET, D], f32r)
        wk_sb = wpool.tile([P, ET, D], f32r)
        wv_sb = wpool.tile([P, ET, D], f32r)
        for w_sb, w_dram in [(wq_sb, wqT), (wk_sb, wkT), (wv_sb, wvT)]:
            nc.sync.dma_start(out=w_sb, in_=w_dram[:, :].rearrange("(t p) e -> p t e", p=P))
        ones_f32 = wpool.tile([P, 2], f32)
        nc.vector.memset(ones_f32, 1.0)
        ones_sb = wpool.tile([P, 2], f32r)
        nc.vector.tensor_copy(out=ones_sb, in_=ones_f32)
        if has_bias:
            bq_sb = wpool.tile([P, D // P], f32)
            bk_sb = wpool.tile([P, D // P], f32)
            bv_sb = wpool.tile([P, D], f32)
            nc.sync.dma_start(out=bq_sb, in_=bq2[:, :])
            nc.sync.dma_start(out=bk_sb, in_=bk2[:, :])
            nc.sync.dma_start(out=bv_sb, in_=bvr[:, :])
        if has_mask:
            mask_sb = wpool.tile([P, S // P], f32)
            nc.sync.dma_start(out=mask_sb, in_=maskf[:, :])

        kt_sb = big.tile([P, ET, S], f32r)
        qt_sb = big.tile([P, ET, SQ], f32r)
        v_sb = big.tile([P, NJB, D], f32r)

        xT_r = xT[:, :].rearrange("(t p) s -> p t s", p=P)
        xqT_r = xqT[:, :].rearrange("(t p) s -> p t s", p=P)

        def body(rep):
            # -- projections (float32r matmuls, fp32 accumulate): Q first so
            # attention can begin while K/V chunks still stream --
            for c in range(SQ // CH):
                xqc = xin.tile([P, ET, CH], f32r, tag="xc", name=f"xq_{rep}_{c}")
                nc.sync.dma_start(out=xqc, in_=xqT_r[:, :, c * CH:(c + 1) * CH])
                for me in range(ET):
                    pq = psum_mm.tile([P, CH], f32, tag="mm512", name=f"pq_{rep}_{c}_{me}")
                    for t in range(ET):
                        nc.tensor.matmul(
                            pq,
                            lhsT=wq_sb[:, t, me * P:(me + 1) * P],
                            rhs=xqc[:, t, :],
                            start=(t == 0), stop=(t == ET - 1))
                    dst = qt_sb[:, me, c * CH:(c + 1) * CH]
                    if has_bias:
                        nc.vector.tensor_scalar_add(dst, pq, bq_sb[:, me:me + 1])
                    else:
                        nc.scalar.copy(out=dst, in_=pq)
            for c in range(S // CH):
                xc = xin.tile([P, ET, CH], f32r, tag="xc", name=f"x_{rep}_{c}")
                nc.sync.dma_start(out=xc, in_=xT_r[:, :, c * CH:(c + 1) * CH])
                for me in range(ET):
                    pk = psum_mm.tile([P, CH], f32, tag="mm512", name=f"pk_{rep}_{c}_{me}")
                    for t in range(ET):
                        nc.tensor.matmul(
                            pk,
                            lhsT=wk_sb[:, t, me * P:(me + 1) * P],
                            rhs=xc[:, t, :],
                            start=(t == 0), stop=(t == ET - 1))
                    dst = kt_sb[:, me, c * CH:(c + 1) * CH]
                    if has_bias:
                        nc.vector.tensor_scalar_add(dst, pk, bk_sb[:, me:me + 1])
                    else:
                        nc.scalar.copy(out=dst, in_=pk)
                for sb_i in range(CH // P):
                    pv = psum_mm.tile([P, D], f32, tag="mm512", name=f"pv_{rep}_{c}_{sb_i}")
                    for t in range(ET):
                        nc.tensor.matmul(
                            pv,
                            lhsT=xc[:, t, sb_i * P:(sb_i + 1) * P],
                            rhs=wv_sb[:, t, :],
                            start=(t == 0), stop=(t == ET - 1))
                    nc.vector.tensor_copy(out=v_sb[:, c * (CH // P) + sb_i, :], in_=pv)

            # -- attention --
            for qt in range(NQT):
                po = [psum_o.tile([P, D], f32, tag=f"po{qs}", name=f"po_{rep}_{qt}_{qs}")
                      for qs in range(NQS)]
                # The four per-subblock denominator groups share one PSUM
                # bank. A start=True matmul clears the WHOLE bank, wiping the
                # other columns' partial sums — so zero the bank once and
                # accumulate with start=False throughout (elements with
                # has_written unset are plain-written, set ones accumulate).
                # f32r matmul dst must have even innermost count and 8B
                # alignment -> write each subblock's denominator twice (N=2).
                psums = psum_sum.tile([P, 2 * NQS], f32, tag="sums", name=f"sums_{rep}_{qt}")
                nc.vector.memset(psums, 0.0)
                for jb in range(NJB):
                    ps_t = psum_mm.tile([P, QTILE], f32, tag="mm512", name=f"ps_{rep}_{qt}_{jb}")
                    for me in range(ET):
                        nc.tensor.matmul(
                            ps_t,
                            lhsT=kt_sb[:, me, jb * P:(jb + 1) * P],
                            rhs=qt_sb[:, me, qt * QTILE:(qt + 1) * QTILE],
                            start=(me == 0), stop=(me == ET - 1))
                    pexp = expp.tile([P, QTILE], f32r, tag="pexp", name=f"pe_{rep}_{qt}_{jb}")
                    nc.scalar.activation(out=pexp, in_=ps_t, func=Exp, scale=inv_sqrt_d)
                    if has_mask:
                        nc.vector.tensor_scalar_mul(pexp, pexp, mask_sb[:, jb:jb + 1])
                    for qs in range(NQS):
                        nc.tensor.matmul(
                            po[qs],
                            lhsT=pexp[:, qs * P:(qs + 1) * P],
                            rhs=v_sb[:, jb, :],
                            start=(jb == 0), stop=(jb == NJB - 1))
                        nc.tensor.matmul(
                            psums[:, 2 * qs:2 * qs + 2],
                            lhsT=pexp[:, qs * P:(qs + 1) * P],
                            rhs=ones_sb,
                            start=False, stop=(jb == NJB - 1),
                            skip_group_check=True)
                recip = smallp.tile([P, 2 * NQS], f32, tag="recip", name=f"rc_{rep}_{qt}")
                nc.vector.reciprocal(out=recip, in_=psums)
                for qs in range(NQS):
                    o_sb = outp.tile([P, D], f32, tag="osb", name=f"o_{rep}_{qt}_{qs}")
                    nc.vector.tensor_scalar_mul(o_sb, po[qs], recip[:, 2 * qs:2 * qs + 1])
                    if has_bias:
                        nc.vector.tensor_add(out=o_sb, in0=o_sb, in1=bv_sb)
                    r0 = (qt * NQS + qs) * P
                    nc.sync.dma_start(out=y[r0:r0 + P, :], in_=o_sb)

        if reps == 1:
            body(0)
        else:
            with tc.For_i(0, reps, 1):
                body(0)
    nc.compile()
    return nc


def _prepare(x, mask, Wq, bq, Wk, bk, Wv, bv):
    """Build (or fetch cached) device program + per-core input maps."""
    x = np.ascontiguousarray(np.asarray(x, dtype=np.float32))
    mask = np.asarray(mask)
    has_bias = bool(np.any(bq) or np.any(bk) or np.any(bv))
    has_mask = bool(np.any(mask))

    key = (has_bias, has_mask)
    if key not in _nc_cache:
        _nc_cache[key] = _build_nc(has_bias, has_mask)
    nc = _nc_cache[key]

    wqT = np.ascontiguousarray(np.asarray(Wq, dtype=np.float32).T)
    wkT = np.ascontiguousarray(np.asarray(Wk, dtype=np.float32).T)
    wvT = np.ascontiguousarray(np.asarray(Wv, dtype=np.float32).T)

    in_maps = []
    for c in range(NCORES):
        b, h = divmod(c, 2)
        xT_b = np.ascontiguousarray(x[b].T)
        m = {
            "xT": xT_b,
            "xqT": np.ascontiguousarray(xT_b[:, h * SQ:(h + 1) * SQ]),
            "wqT": wqT, "wkT": wkT, "wvT": wvT,
        }
        if has_bias:
            m["bq2"] = np.ascontiguousarray(
                np.asarray(bq, np.float32).reshape(D // P, P).T)
            m["bk2"] = np.ascontiguousarray(
                np.asarray(bk, np.float32).reshape(D // P, P).T)
            m["bvr"] = np.ascontiguousarray(
                np.broadcast_to(np.asarray(bv, np.float32), (P, D)))
        if has_mask:
            keep = 1.0 - mask[b].astype(np.float32)
            m["maskf"] = np.ascontiguousarray(keep.reshape(S // P, P).T)
        in_maps.append(m)
    return nc, in_maps


def _gather(res):
    out = np.empty((B, S, D), dtype=np.float32)
    for c in range(NCORES):
        b, h = divmod(c, 2)
        out[b, h * SQ:(h + 1) * SQ, :] = res.results[c]["y"]
    return out


def kernel(x, mask, Wq, bq, Wk, bk, Wv, bv):
    global last_results
    from concourse.bass_utils import run_bass_kernel_spmd

    nc, in_maps = _prepare(x, mask, Wq, bq, Wk, bk, Wv, bv)
    res = run_bass_kernel_spmd(nc, in_maps, core_ids=list(range(NCORES)))
    last_results = res
    return _gather(res)


# revision 15
# speedup vs baseline: 1.3416x; 1.3416x over previous
"""Single-head full attention (B=4, S=4096, D=512) on 8 TRN2 NeuronCores.

Sharding: core c handles batch b = c//2, query half h = c%2 (2048 queries).
Each core projects K/V for its batch's full sequence (recompute across the
core pair — cheaper than a collective), projects its query half, then runs
a flash-style attention entirely on-chip.

Device layouts (per core):
  xT   [512, 4096]  = x[b].T           (host pre-transposed)
  xqT  [512, 2048]  = query-half cols of xT
  wqT/wkT/wvT [512, 512] = W.T         (host pre-transposed)
  y    [2048, 512]  = output rows for this core's query half

On-chip (partition index p, 128 partitions):
  kt_sb [128, 4, 4096] f32 : K^T, partition p + tile me -> e = me*128+p
  qt_sb [128, 4, 2048] f32 : Q^T, same e layout
  v_sb  [128, 32, 512] bf16: V natural, partition p + block jb -> j = jb*128+p
Scores are computed transposed (S^T[j, q]) so exp(S^T) blocks serve directly
as the stationary operand of the P@V matmul, producing O in natural [q, d]
orientation; softmax denominators come from an N=1 ones-matmul sharing the
same stationary tile. Matmuls run as float32r (full PE rate at N>=512) for
projections and scores; P@V runs bf16. exp is scalar-engine with the 1/sqrt(D)
scale fused; no max-subtraction (scores are provably O(1) for this problem:
softmax is shift-invariant, exp stays in fp32 range).
"""
import math
import os
import numpy as np

B, S, D = 4, 4096, 512
P = 128
SQ = S // 2          # queries per core
NCORES = 8
QTILE = 512          # query columns per score/PV pass
CH = 512             # projection chunk (sequence columns per x chunk)

last_results = None  # BassKernelResults of the most recent run (for test.py)

_nc_cache = {}


def _build_nc(has_bias, has_mask, reps=1):
    import concourse.bacc as bacc
    import concourse.tile as tile
    from concourse import mybir
    from contextlib import ExitStack

    f32 = mybir.dt.float32
    f16 = mybir.dt.float16
    Exp = mybir.ActivationFunctionType.Exp

    nc = bacc.Bacc("TRN2", target_bir_lowering=False, debug=False)
    xT = nc.declare_dram_parameter("xT", [D, S], f16, False)
    xqT = nc.declare_dram_parameter("xqT", [D, SQ], f16, False)
    wqT = nc.declare_dram_parameter("wqT", [D, D], f16, False)
    wkT = nc.declare_dram_parameter("wkT", [D, D], f16, False)
    wvT = nc.declare_dram_parameter("wvT", [D, D], f16, False)
    if has_bias:
        bq2 = nc.declare_dram_parameter("bq2", [P, D // P], f32, False)
        bk2 = nc.declare_dram_parameter("bk2", [P, D // P], f32, False)
        bvr = nc.declare_dram_parameter("bvr", [P, D], f32, False)
    if has_mask:
        maskf = nc.declare_dram_parameter("maskf", [P, S // P], f32, False)
    y = nc.declare_dram_parameter("y", [SQ, D], f32, True)

    ET = D // P          # 4 e-tiles
    NJB = S // P         # 32 key blocks
    NQT = SQ // QTILE    # 4 query tiles
    NQS = QTILE // P     # 4 query subblocks per tile
    inv_sqrt_d = 1.0 / math.sqrt(D)

    with tile.TileContext(nc) as tc, ExitStack() as ctx:
        wpool = ctx.enter_context(tc.tile_pool(name="wpool", bufs=1))
        big = ctx.enter_context(tc.tile_pool(name="big", bufs=1))
        xin = ctx.enter_context(tc.tile_pool(name="xin", bufs=2 if (has_bias or has_mask) else 3))
        expp = ctx.enter_context(tc.tile_pool(name="expp", bufs=3))
        outp = ctx.enter_context(tc.tile_pool(name="outp", bufs=2))
        smallp = ctx.enter_context(tc.tile_pool(name="smallp", bufs=2))
        # PSUM: one shared [128,512] accumulate tag (projections + scores)
        # keeps every phase inside 8 banks: 3 (mm512) + 4 (po) + 1 (sums).
        psum_mm = ctx.enter_context(tc.tile_pool(name="psum_mm", bufs=3, space="PSUM"))
        psum_o = ctx.enter_context(tc.tile_pool(name="psum_o", bufs=1, space="PSUM"))
        psum_sum = ctx.enter_context(tc.tile_pool(name="psum_sum", bufs=1, space="PSUM"))

        wq_sb = wpool.tile([P, ET, D], f16)
        wk_sb = wpool.tile([P, ET, D], f16)
        wv_sb = wpool.tile([P, ET, D], f16)
        for w_sb, w_dram in [(wq_sb, wqT), (wk_sb, wkT), (wv_sb, wvT)]:
            nc.sync.dma_start(out=w_sb, in_=w_dram[:, :].rearrange("(t p) e -> p t e", p=P))
        ones_sb = wpool.tile([P, 2], f16)
        nc.vector.memset(ones_sb, 1.0)
        if has_bias:
            bq_sb = wpool.tile([P, D // P], f32)
            bk_sb = wpool.tile([P, D // P], f32)
            bv_sb = wpool.tile([P, D], f32)
            nc.sync.dma_start(out=bq_sb, in_=bq2[:, :])
            nc.sync.dma_start(out=bk_sb, in_=bk2[:, :])
            nc.sync.dma_start(out=bv_sb, in_=bvr[:, :])
        if has_mask:
            mask_sb = wpool.tile([P, S // P], f32)
            nc.sync.dma_start(out=mask_sb, in_=maskf[:, :])

        kt_sb = big.tile([P, ET, S], f16)
        qt_sb = big.tile([P, ET, SQ], f16)
        v_sb = big.tile([P, NJB, D], f16)

        xT_r = xT[:, :].rearrange("(t p) s -> p t s", p=P)
        xqT_r = xqT[:, :].rearrange("(t p) s -> p t s", p=P)

        def body(rep):
            # -- projections (float32r matmuls, fp32 accumulate): Q first so
            # attention can begin while K/V chunks still stream --
            for c in range(SQ // CH):
                xqc = xin.tile([P, ET, CH], f16, tag="xc", name=f"xq_{rep}_{c}")
                nc.sync.dma_start(out=xqc, in_=xqT_r[:, :, c * CH:(c + 1) * CH])
                for me in range(ET):
                    pq = psum_mm.tile([P, CH], f32, tag="mm512", name=f"pq_{rep}_{c}_{me}")
                    for t in range(ET):
                        nc.tensor.matmul(
                            pq,
                            lhsT=wq_sb[:, t, me * P:(me + 1) * P],
                            rhs=xqc[:, t, :],
                            start=(t == 0), stop=(t == ET - 1))
                    dst = qt_sb[:, me, c * CH:(c + 1) * CH]
                    if has_bias:
                        nc.vector.tensor_scalar_add(dst, pq, bq_sb[:, me:me + 1])
                    else:
                        nc.scalar.copy(out=dst, in_=pq)
            for c in range(S // CH):
                xc = xin.tile([P, ET, CH], f16, tag="xc", name=f"x_{rep}_{c}")
                nc.sync.dma_start(out=xc, in_=xT_r[:, :, c * CH:(c + 1) * CH])
                for me in range(ET):
                    pk = psum_mm.tile([P, CH], f32, tag="mm512", name=f"pk_{rep}_{c}_{me}")
                    for t in range(ET):
                        nc.tensor.matmul(
                            pk,
                            lhsT=wk_sb[:, t, me * P:(me + 1) * P],
                            rhs=xc[:, t, :],
                            start=(t == 0), stop=(t == ET - 1))
                    dst = kt_sb[:, me, c * CH:(c + 1) * CH]
                    if has_bias:
                        nc.vector.tensor_scalar_add(dst, pk, bk_sb[:, me:me + 1])
                    else:
                        nc.scalar.copy(out=dst, in_=pk)
                for sb_i in range(CH // P):
                    pv = psum_mm.tile([P, D], f32, tag="mm512", name=f"pv_{rep}_{c}_{sb_i}")
                    for t in range(ET):
                        nc.tensor.matmul(
                            pv,
                            lhsT=xc[:, t, sb_i * P:(sb_i + 1) * P],
                            rhs=wv_sb[:, t, :],
                            start=(t == 0), stop=(t == ET - 1))
                    nc.vector.tensor_copy(out=v_sb[:, c * (CH // P) + sb_i, :], in_=pv)

            # -- attention --
            for qt in range(NQT):
                po = [psum_o.tile([P, D], f32, tag=f"po{qs}", name=f"po_{rep}_{qt}_{qs}")
                      for qs in range(NQS)]
                # The four per-subblock denominator groups share one PSUM
                # bank. A start=True matmul clears the WHOLE bank, wiping the
                # other columns' partial sums — so zero the bank once and
                # accumulate with start=False throughout (elements with
                # has_written unset are plain-written, set ones accumulate).
                psums = psum_sum.tile([P, 2 * NQS], f32, tag="sums", name=f"sums_{rep}_{qt}")
                nc.vector.memset(psums, 0.0)
                for jb in range(NJB):
                    ps_t = psum_mm.tile([P, QTILE], f32, tag="mm512", name=f"ps_{rep}_{qt}_{jb}")
                    for me in range(ET):
                        nc.tensor.matmul(
                            ps_t,
                            lhsT=kt_sb[:, me, jb * P:(jb + 1) * P],
                            rhs=qt_sb[:, me, qt * QTILE:(qt + 1) * QTILE],
                            start=(me == 0), stop=(me == ET - 1))
                    pexp = expp.tile([P, QTILE], f16, tag="pexp", name=f"pe_{rep}_{qt}_{jb}")
                    nc.scalar.activation(out=pexp, in_=ps_t, func=Exp, scale=inv_sqrt_d)
                    if has_mask:
                        nc.vector.tensor_scalar_mul(pexp, pexp, mask_sb[:, jb:jb + 1])
                    for qs in range(NQS):
                        nc.tensor.matmul(
                            po[qs],
                            lhsT=pexp[:, qs * P:(qs + 1) * P],
                            rhs=v_sb[:, jb, :],
                            start=(jb == 0), stop=(jb == NJB - 1))
                        nc.tensor.matmul(
                            psums[:, 2 * qs:2 * qs + 2],
                            lhsT=pexp[:, qs * P:(qs + 1) * P],
                            rhs=ones_sb,
                            start=False, stop=(jb == NJB - 1),
                            skip_group_check=True)
                recip = smallp.tile([P, 2 * NQS], f32, tag="recip", name=f"rc_{rep}_{qt}")
                nc.vector.reciprocal(out=recip, in_=psums)
                for qs in range(NQS):
                    o_sb = outp.tile([P, D], f32, tag="osb", name=f"o_{rep}_{qt}_{qs}")
                    nc.vector.tensor_scalar_mul(o_sb, po[qs], recip[:, 2 * qs:2 * qs + 1])
                    if has_bias:
                        nc.vector.tensor_add(out=o_sb, in0=o_sb, in1=bv_sb)
                    r0 = (qt * NQS + qs) * P
                    nc.sync.dma_start(out=y[r0:r0 + P, :], in_=o_sb)

        if reps == 1:
            body(0)
        else:
            with tc.For_i(0, reps, 1):
                body(0)
    nc.compile()
    return nc


def _prepare(x, mask, Wq, bq, Wk, bk, Wv, bv):
    """Build (or fetch cached) device program + per-core input maps."""
    x = np.asarray(x, dtype=np.float32)
    mask = np.asarray(mask)
    has_bias = bool(np.any(bq) or np.any(bk) or np.any(bv))
    has_mask = bool(np.any(mask))

    key = (has_bias, has_mask)
    if key not in _nc_cache:
        _nc_cache[key] = _build_nc(has_bias, has_mask)
    nc = _nc_cache[key]

    wqT = np.ascontiguousarray(np.asarray(Wq, dtype=np.float32).T.astype(np.float16))
    wkT = np.ascontiguousarray(np.asarray(Wk, dtype=np.float32).T.astype(np.float16))
    wvT = np.ascontiguousarray(np.asarray(Wv, dtype=np.float32).T.astype(np.float16))

    in_maps = []
    for c in range(NCORES):
        b, h = divmod(c, 2)
        xT_b = np.ascontiguousarray(x[b].T.astype(np.float16))
        m = {
            "xT": xT_b,
            "xqT": np.ascontiguousarray(xT_b[:, h * SQ:(h + 1) * SQ]),
            "wqT": wqT, "wkT": wkT, "wvT": wvT,
        }
        if has_bias:
            m["bq2"] = np.ascontiguousarray(
                np.asarray(bq, np.float32).reshape(D // P, P).T)
            m["bk2"] = np.ascontiguousarray(
                np.asarray(bk, np.float32).reshape(D // P, P).T)
            m["bvr"] = np.ascontiguousarray(
                np.broadcast_to(np.asarray(bv, np.float32), (P, D)))
        if has_mask:
            keep = 1.0 - mask[b].astype(np.float32)
            m["maskf"] = np.ascontiguousarray(keep.reshape(S // P, P).T)
        in_maps.append(m)
    return nc, in_maps


def _gather(res):
    out = np.empty((B, S, D), dtype=np.float32)
    for c in range(NCORES):
        b, h = divmod(c, 2)
        out[b, h * SQ:(h + 1) * SQ, :] = res.results[c]["y"]
    return out


def kernel(x, mask, Wq, bq, Wk, bk, Wv, bv):
    global last_results
    from concourse.bass_utils import run_bass_kernel_spmd

    nc, in_maps = _prepare(x, mask, Wq, bq, Wk, bk, Wv, bv)
    res = run_bass_kernel_spmd(nc, in_maps, core_ids=list(range(NCORES)))
    last_results = res
    return _gather(res)


# revision 16
# speedup vs baseline: 1.5580x; 1.1613x over previous
"""Single-head full attention (B=4, S=4096, D=512) on 8 TRN2 NeuronCores.

Sharding: core c handles batch b = c//2, query half h = c%2 (2048 queries).

Key algebraic fold: scores = (x_q Wq^T)(x Wk^T)^T / sqrt(D)
                           = x_q @ M @ x^T,   M = Wq^T Wk / sqrt(D)  (host).
So K is never materialized: x^T itself (resident in SBUF, fp16) is the
stationary operand of the scores matmul, and T = x_q @ M replaces Q.
Per-query additive terms drop out of softmax (row-shift invariance); with
biases the per-key additive beta[j] = (bq Wk/sqrt(D))x[j]^T is applied as a
multiplier exp(beta) on the exp'd scores (the bq.bk constant cancels).

Device layouts (per core, fp16 operands, fp32 accumulate):
  xt_sb [128, 4, 4096]: x^T, partition p + tile t -> d' = t*128+p
  xq_sb [128, 4, 2048]: query-half columns of x^T (same layout)
  tt_sb [128, 4, 2048]: T^T = (x_q @ M)^T
  v_sb  [128, 32, 512]: V natural, partition p + block jb -> j = jb*128+p
Scores are computed transposed (S^T[j, q]) so exp(S^T) blocks serve directly
as the stationary operand of the P@V matmul, producing O in natural [q, d]
orientation. Softmax denominators come from an N=2 ones-matmul sharing the
same stationary tile (denominator written twice); the four per-subblock
denominator groups share one PSUM bank, so the bank is zeroed once and all
groups accumulate with start=False (a start=True matmul clears the whole
bank). No max-subtraction: scores are O(1) here and softmax is
shift-invariant, exp stays comfortably in fp32/fp16 range.
"""
import math
import numpy as np

B, S, D = 4, 4096, 512
P = 128
SQ = S // 2          # queries per core
NCORES = 8
QTILE = 512          # query columns per score/PV pass

last_results = None  # BassKernelResults of the most recent run (for test.py)

_nc_cache = {}


def _build_nc(has_bias, has_mask, reps=1):
    import concourse.bacc as bacc
    import concourse.tile as tile
    from concourse import mybir
    from contextlib import ExitStack

    f32 = mybir.dt.float32
    f16 = mybir.dt.float16
    Exp = mybir.ActivationFunctionType.Exp

    nc = bacc.Bacc("TRN2", target_bir_lowering=False, debug=False)
    xT = nc.declare_dram_parameter("xT", [D, S], f16, False)
    xqT = nc.declare_dram_parameter("xqT", [D, SQ], f16, False)
    mT = nc.declare_dram_parameter("mT", [D, D], f16, False)
    wvT = nc.declare_dram_parameter("wvT", [D, D], f16, False)
    if has_bias:
        wtl = nc.declare_dram_parameter("wtl", [P, D // P], f16, False)
        bvr = nc.declare_dram_parameter("bvr", [P, D], f32, False)
    if has_mask:
        maskf = nc.declare_dram_parameter("maskf", [P, S // P], f32, False)
    y = nc.declare_dram_parameter("y", [SQ, D], f32, True)

    ET = D // P          # 4 d'-tiles
    NJB = S // P         # 32 key blocks
    NQT = SQ // QTILE    # 4 query tiles
    NQS = QTILE // P     # 4 query subblocks per tile

    with tile.TileContext(nc) as tc, ExitStack() as ctx:
        wpool = ctx.enter_context(tc.tile_pool(name="wpool", bufs=1))
        big = ctx.enter_context(tc.tile_pool(name="big", bufs=1))
        expp = ctx.enter_context(tc.tile_pool(name="expp", bufs=4))
        outp = ctx.enter_context(tc.tile_pool(name="outp", bufs=3))
        smallp = ctx.enter_context(tc.tile_pool(name="smallp", bufs=2))
        # PSUM: shared [128,512] accumulate tag (projections + scores) keeps
        # every phase inside 8 banks: 3 (mm512) + 4 (po) + 1 (sums).
        psum_mm = ctx.enter_context(tc.tile_pool(name="psum_mm", bufs=3, space="PSUM"))
        psum_o = ctx.enter_context(tc.tile_pool(name="psum_o", bufs=1, space="PSUM"))
        psum_sum = ctx.enter_context(tc.tile_pool(name="psum_sum", bufs=1, space="PSUM"))

        m_sb = wpool.tile([P, ET, D], f16)
        wv_sb = wpool.tile([P, ET, D], f16)
        nc.sync.dma_start(out=m_sb, in_=mT[:, :].rearrange("(t p) e -> p t e", p=P))
        nc.sync.dma_start(out=wv_sb, in_=wvT[:, :].rearrange("(t p) e -> p t e", p=P))
        ones_sb = wpool.tile([P, 2], f16)
        nc.vector.memset(ones_sb, 1.0)
        if has_bias:
            wtl_sb = wpool.tile([P, D // P], f16)
            bv_sb = wpool.tile([P, D], f32)
            nc.sync.dma_start(out=wtl_sb, in_=wtl[:, :])
            nc.sync.dma_start(out=bv_sb, in_=bvr[:, :])
        if has_mask:
            mask_sb = wpool.tile([P, S // P], f32)
            nc.sync.dma_start(out=mask_sb, in_=maskf[:, :])

        xt_sb = big.tile([P, ET, S], f16)
        xq_sb = big.tile([P, ET, SQ], f16)
        tt_sb = big.tile([P, ET, SQ], f16)
        v_sb = big.tile([P, NJB, D], f16)

        xT_r = xT[:, :].rearrange("(t p) s -> p t s", p=P)
        xqT_r = xqT[:, :].rearrange("(t p) s -> p t s", p=P)

        def body(rep):
            # resident x^T / x_q^T loads, chunked so consumers unlock early
            for c in range(SQ // QTILE):
                nc.sync.dma_start(
                    out=xq_sb[:, :, c * QTILE:(c + 1) * QTILE],
                    in_=xqT_r[:, :, c * QTILE:(c + 1) * QTILE])
            for c in range(S // QTILE):
                nc.sync.dma_start(
                    out=xt_sb[:, :, c * QTILE:(c + 1) * QTILE],
                    in_=xT_r[:, :, c * QTILE:(c + 1) * QTILE])

            # T^T projection: M-stationary, x_q^T-moving
            for c in range(SQ // QTILE):
                for me in range(ET):
                    pq = psum_mm.tile([P, QTILE], f32, tag="mm512",
                                      name=f"pq_{rep}_{c}_{me}")
                    for t in range(ET):
                        nc.tensor.matmul(
                            pq,
                            lhsT=m_sb[:, t, me * P:(me + 1) * P],
                            rhs=xq_sb[:, t, c * QTILE:(c + 1) * QTILE],
                            start=(t == 0), stop=(t == ET - 1))
                    nc.scalar.copy(out=tt_sb[:, me, c * QTILE:(c + 1) * QTILE], in_=pq)

            # V projection: x^T-stationary, Wv^T-moving
            for sb_i in range(NJB):
                pv = psum_mm.tile([P, D], f32, tag="mm512", name=f"pv_{rep}_{sb_i}")
                for t in range(ET):
                    nc.tensor.matmul(
                        pv,
                        lhsT=xt_sb[:, t, sb_i * P:(sb_i + 1) * P],
                        rhs=wv_sb[:, t, :],
                        start=(t == 0), stop=(t == ET - 1))
                nc.vector.tensor_copy(out=v_sb[:, sb_i, :], in_=pv)

            # per-key bias multiplier exp(beta[j]) (only when biases present)
            if has_bias:
                bmul_sb = smallp.tile([P, NJB], f32, tag="bmul", name=f"bm_{rep}")
                for jb in range(NJB):
                    pb = psum_sum.tile([P, 2], f32, tag="bsum", name=f"pb_{rep}_{jb}")
                    for t in range(ET):
                        nc.tensor.matmul(
                            pb,
                            lhsT=xt_sb[:, t, jb * P:(jb + 1) * P],
                            rhs=wtl_sb[:, t:t + 1].to_broadcast([P, 2]),
                            start=(t == 0), stop=(t == ET - 1))
                    nc.scalar.activation(out=bmul_sb[:, jb:jb + 1], in_=pb[:, 0:1],
                                         func=Exp, scale=1.0)

            # attention
            for qt in range(NQT):
                po = [psum_o.tile([P, D], f32, tag=f"po{qs}", name=f"po_{rep}_{qt}_{qs}")
                      for qs in range(NQS)]
                psums = psum_sum.tile([P, 2 * NQS], f32, tag="sums",
                                      name=f"sums_{rep}_{qt}")
                nc.vector.memset(psums, 0.0)
                for jb in range(NJB):
                    ps_t = psum_mm.tile([P, QTILE], f32, tag="mm512",
                                        name=f"ps_{rep}_{qt}_{jb}")
                    for t in range(ET):
                        nc.tensor.matmul(
                            ps_t,
                            lhsT=xt_sb[:, t, jb * P:(jb + 1) * P],
                            rhs=tt_sb[:, t, qt * QTILE:(qt + 1) * QTILE],
                            start=(t == 0), stop=(t == ET - 1))
                    pexp = expp.tile([P, QTILE], f16, tag="pexp",
                                     name=f"pe_{rep}_{qt}_{jb}")
                    nc.scalar.activation(out=pexp, in_=ps_t, func=Exp, scale=1.0)
                    if has_bias:
                        nc.vector.tensor_scalar_mul(pexp, pexp, bmul_sb[:, jb:jb + 1])
                    if has_mask:
                        nc.vector.tensor_scalar_mul(pexp, pexp, mask_sb[:, jb:jb + 1])
                    for qs in range(NQS):
                        nc.tensor.matmul(
                            po[qs],
                            lhsT=pexp[:, qs * P:(qs + 1) * P],
                            rhs=v_sb[:, jb, :],
                            start=(jb == 0), stop=(jb == NJB - 1))
                        nc.tensor.matmul(
                            psums[:, 2 * qs:2 * qs + 2],
                            lhsT=pexp[:, qs * P:(qs + 1) * P],
                            rhs=ones_sb,
                            start=False, stop=(jb == NJB - 1),
                            skip_group_check=True)
                recip = smallp.tile([P, 2 * NQS], f32, tag="recip", name=f"rc_{rep}_{qt}")
                nc.vector.reciprocal(out=recip, in_=psums)
                for qs in range(NQS):
                    o_sb = outp.tile([P, D], f32, tag="osb", name=f"o_{rep}_{qt}_{qs}")
                    nc.vector.tensor_scalar_mul(o_sb, po[qs], recip[:, 2 * qs:2 * qs + 1])
                    if has_bias:
                        nc.vector.tensor_add(out=o_sb, in0=o_sb, in1=bv_sb)
                    r0 = (qt * NQS + qs) * P
                    nc.sync.dma_start(out=y[r0:r0 + P, :], in_=o_sb)

        if reps == 1:
            body(0)
        else:
            with tc.For_i(0, reps, 1):
                body(0)
    nc.compile()
    return nc


def _prepare(x, mask, Wq, bq, Wk, bk, Wv, bv):
    """Build (or fetch cached) device program + per-core input maps."""
    x = np.asarray(x, dtype=np.float32)
    mask = np.asarray(mask)
    Wq = np.asarray(Wq, dtype=np.float32)
    Wk = np.asarray(Wk, dtype=np.float32)
    Wv = np.asarray(Wv, dtype=np.float32)
    bq = np.asarray(bq, dtype=np.float32)
    bk = np.asarray(bk, dtype=np.float32)
    bv = np.asarray(bv, dtype=np.float32)
    has_bias = bool(np.any(bq) or np.any(bk) or np.any(bv))
    has_mask = bool(np.any(mask))

    key = (has_bias, has_mask)
    if key not in _nc_cache:
        _nc_cache[key] = _build_nc(has_bias, has_mask)
    nc = _nc_cache[key]

    inv_sqrt_d = 1.0 / math.sqrt(D)
    M = (Wq.T.astype(np.float64) @ Wk.astype(np.float64)) * inv_sqrt_d
    mT_h = np.ascontiguousarray(M.astype(np.float32).astype(np.float16))
    wvT_h = np.ascontiguousarray(Wv.T.astype(np.float16))

    in_maps = []
    for c in range(NCORES):
        b, h = divmod(c, 2)
        xT_b = np.ascontiguousarray(x[b].T.astype(np.float16))
        m = {
            "xT": xT_b,
            "xqT": np.ascontiguousarray(xT_b[:, h * SQ:(h + 1) * SQ]),
            "mT": mT_h, "wvT": wvT_h,
        }
        if has_bias:
            # per-key additive beta[j] = (bq Wk/sqrt(D)).x[j]; the bq.bk
            # constant shifts all keys equally and cancels in softmax.
            wt = (bq @ Wk) * inv_sqrt_d              # [D]
            m["wtl"] = np.ascontiguousarray(
                wt.reshape(D // P, P).T.astype(np.float16))
            m["bvr"] = np.ascontiguousarray(np.broadcast_to(bv, (P, D))).copy()
        if has_mask:
            keep = 1.0 - mask[b].astype(np.float32)
            m["maskf"] = np.ascontiguousarray(keep.reshape(S // P, P).T)
        in_maps.append(m)
    return nc, in_maps


def _gather(res):
    out = np.empty((B, S, D), dtype=np.float32)
    for c in range(NCORES):
        b, h = divmod(c, 2)
        out[b, h * SQ:(h + 1) * SQ, :] = res.results[c]["y"]
    return out


def kernel(x, mask, Wq, bq, Wk, bk, Wv, bv):
    global last_results
    from concourse.bass_utils import run_bass_kernel_spmd

    nc, in_maps = _prepare(x, mask, Wq, bq, Wk, bk, Wv, bv)
    res = run_bass_kernel_spmd(nc, in_maps, core_ids=list(range(NCORES)))
    last_results = res
    return _gather(res)
